# revision 3
# baseline (speedup 1.0000x reference)
"""Trainium2 Bass kernel: batched majority-vote (REP=2) BCH fallback decode.

Input : received_bits (65536, 2000) float32 soft bits in [0, 1)
Output: decoded (65536, 1000) float32, error_count (65536,) float32

Math (REP=2 groups of consecutive bits):
  hard = received > 0.5
  decoded[j]  = hard[2j] AND hard[2j+1]
  error_count = #{j : hard[2j] XOR hard[2j+1]}
              = sum(hard) - 2 * sum(decoded)        # the identity used below

Sharding: pure data parallel over the batch dim across 8 NeuronCores
(8192 rows per core). Per core, rows are tiled 128 at a time onto SBUF
partitions; DVE does one tensor_scalar (threshold, accum -> per-row
sum S) and one scalar_tensor_tensor (AND of even/odd columns, accum ->
per-row sum D) per tile; error counts come from one final S - 2D op.
"""

import os

import numpy as np

import concourse.bass as bass
import concourse.bacc as bacc
import concourse.mybir as mybir
from concourse.bass_utils import run_bass_kernel_spmd
from concourse.tile import TileContext

F32 = mybir.dt.float32
P = 128
B = 65536
C = 2000
M = C // 2
N_CORES = 8
ROWS = B // N_CORES  # 8192 rows per core


def build_nc(rows: int = ROWS, bufs: int = 4) -> bass.Bass:
    assert rows % P == 0
    nt = rows // P

    nc = bacc.Bacc("TRN2", target_bir_lowering=False, debug=False)
    x = nc.dram_tensor("x", [rows, C], F32, kind="ExternalInput")
    dec = nc.dram_tensor("dec", [rows, M], F32, kind="ExternalOutput")
    ec = nc.dram_tensor("ec", [P, nt], F32, kind="ExternalOutput")

    with TileContext(nc) as tc:
        with (
            tc.tile_pool(name="io", bufs=bufs) as pool,
            tc.tile_pool(name="acc", bufs=1) as accpool,
        ):
            s_acc = accpool.tile([P, nt], F32, name="s_acc")
            d_acc = accpool.tile([P, nt], F32, name="d_acc")
            ec_t = accpool.tile([P, nt], F32, name="ec_t")
            for t in range(nt):
                xt = pool.tile([P, C], F32, tag="x", name=f"x_{t}")
                nc.sync.dma_start(out=xt[:], in_=x[t * P : (t + 1) * P, :])
                hard = pool.tile([P, C], F32, tag="h", name=f"h_{t}")
                # hard = (x > 0.5); op1=add is the accum reduce op:
                # s_acc[:, t] = per-row sum of hard
                nc.vector.tensor_scalar(
                    hard[:],
                    xt[:],
                    0.5,
                    None,
                    mybir.AluOpType.is_gt,
                    mybir.AluOpType.add,
                    accum_out=s_acc[:, t : t + 1],
                )
                dect = pool.tile([P, M], F32, tag="d", name=f"d_{t}")
                # Pre-touch the slot: the STT ISA format has a single
                # sync-wait slot, so the slot-release DMA wait must land
                # on this memset instead (codegen rejects 2 waits on STT).
                nc.vector.memset(dect[:1, :1], 0.0)
                # dec = hard_even AND hard_odd; d_acc[:, t] = per-row sum
                nc.vector.scalar_tensor_tensor(
                    dect[:],
                    hard[:, 0:C:2],
                    0.0,
                    hard[:, 1:C:2],
                    mybir.AluOpType.bypass,
                    mybir.AluOpType.logical_and,
                    accum_out=d_acc[:, t : t + 1],
                )
                nc.sync.dma_start(out=dec[t * P : (t + 1) * P, :], in_=dect[:])
            # ec = S - 2*D
            nc.vector.scalar_tensor_tensor(
                ec_t[:],
                d_acc[:],
                -2.0,
                s_acc[:],
                mybir.AluOpType.mult,
                mybir.AluOpType.add,
            )
            nc.sync.dma_start(out=ec[:, :], in_=ec_t[:])
    nc.compile()
    return nc


_CACHE: dict = {}


def _run(x: np.ndarray, trace: bool = False):
    if "nc" not in _CACHE:
        _CACHE["nc"] = build_nc()
    nc = _CACHE["nc"]
    in_maps = [
        {"x": np.ascontiguousarray(x[i * ROWS : (i + 1) * ROWS])}
        for i in range(N_CORES)
    ]
    return run_bass_kernel_spmd(
        nc, in_maps, core_ids=list(range(N_CORES)), trace=trace
    )


def kernel(received_bits: np.ndarray):
    x = np.asarray(received_bits, dtype=np.float32)
    assert x.shape == (B, C), x.shape
    trace = bool(os.environ.get("KERNEL_TRACE"))
    res = _run(x, trace=trace)
    _CACHE["last_result"] = res
    decoded = np.concatenate([r["dec"] for r in res.results], axis=0)
    ec = np.concatenate(
        [r["ec"].T.reshape(-1) for r in res.results], axis=0
    )
    return decoded, ec


# revision 4
# speedup vs baseline: 1.2813x; 1.2813x over previous
"""Trainium2 Bass kernel: batched majority-vote (REP=2) BCH fallback decode.

Input : received_bits (65536, 2000) float32 soft bits in [0, 1)
Output: decoded (65536, 1000) float32, error_count (65536,) float32

Math (REP=2 groups of consecutive bits):
  hard = received > 0.5
  decoded[j]  = hard[2j] AND hard[2j+1]
  error_count = #{j : hard[2j] XOR hard[2j+1]}
              = sum(hard) - 2 * sum(decoded)       # identity used below

Sharding: pure data parallel over the batch across 8 NeuronCores (8192
rows per core). Per core, tiles of 256 rows (2 rows per SBUF partition,
16 KB contiguous HBM runs per partition); DVE does one tensor_scalar
(threshold, accum -> per-row sum S) and one scalar_tensor_tensor (AND
of even/odd columns, accum -> per-row sum D) per row-half; error counts
come from one final S - 2D op. Input loads use the Sync HWDGE ring,
output stores the Scalar/ACT ring.
"""

import os

import numpy as np

import concourse.bacc as bacc
import concourse.bass as bass
import concourse.mybir as mybir
from concourse.bass_utils import run_bass_kernel_spmd
from concourse.tile import TileContext

F32 = mybir.dt.float32
P = 128
B = 65536
C = 2000
M = C // 2
RPP = 2
TROWS = P * RPP  # 256 rows per tile
N_CORES = 8
ROWS = B // N_CORES  # 8192 rows per core


def build_nc(rows: int = ROWS, bufs_x: int = 4, bufs_h: int = 2, bufs_d: int = 3) -> bass.Bass:
    assert rows % TROWS == 0
    nt = rows // TROWS
    ncols = nt * RPP  # accumulator columns == rows/128

    nc = bacc.Bacc("TRN2", target_bir_lowering=False, debug=False)
    x = nc.dram_tensor("x", [rows, C], F32, kind="ExternalInput")
    dec = nc.dram_tensor("dec", [rows, M], F32, kind="ExternalOutput")
    ec = nc.dram_tensor("ec", [P, ncols], F32, kind="ExternalOutput")

    # tile t, partition p  <->  rows t*256 + 2p + {0,1}
    x_t = x.rearrange("(t p two) c -> t p (two c)", p=P, two=RPP)
    dec_t = dec.rearrange("(t p two) m -> t p (two m)", p=P, two=RPP)

    with TileContext(nc) as tc:
        with (
            tc.tile_pool(name="io", bufs=2) as pool,
            tc.tile_pool(name="acc", bufs=1) as accpool,
        ):
            s_acc = accpool.tile([P, ncols], F32, name="s_acc")
            d_acc = accpool.tile([P, ncols], F32, name="d_acc")
            ec_t = accpool.tile([P, ncols], F32, name="ec_t")
            for t in range(nt):
                xt = pool.tile([P, RPP * C], F32, tag="x", name=f"x_{t}", bufs=bufs_x)
                nc.sync.dma_start(out=xt[:], in_=x_t[t])
                hard = pool.tile([P, RPP * C], F32, tag="h", name=f"h_{t}", bufs=bufs_h)
                dect = pool.tile([P, RPP * M], F32, tag="d", name=f"d_{t}", bufs=bufs_d)
                for k in range(RPP):
                    a = t * RPP + k
                    # hard = (x > 0.5); op1=add is the accum reduce op
                    nc.vector.tensor_scalar(
                        hard[:, k * C : (k + 1) * C],
                        xt[:, k * C : (k + 1) * C],
                        0.5,
                        None,
                        mybir.AluOpType.is_gt,
                        mybir.AluOpType.add,
                        accum_out=s_acc[:, a : a + 1],
                    )
                    # dec = hard_even AND hard_odd, accum -> per-row sum D
                    nc.vector.scalar_tensor_tensor(
                        dect[:, k * M : (k + 1) * M],
                        hard[:, k * C : (k + 1) * C : 2],
                        0.0,
                        hard[:, k * C + 1 : (k + 1) * C : 2],
                        mybir.AluOpType.bypass,
                        mybir.AluOpType.logical_and,
                        accum_out=d_acc[:, a : a + 1],
                    )
                nc.scalar.dma_start(out=dec_t[t], in_=dect[:])
            # ec = S - 2*D
            nc.vector.scalar_tensor_tensor(
                ec_t[:],
                d_acc[:],
                -2.0,
                s_acc[:],
                mybir.AluOpType.mult,
                mybir.AluOpType.add,
            )
            nc.scalar.dma_start(out=ec[:, :], in_=ec_t[:])
    nc.compile()
    return nc


_CACHE: dict = {}


def _run(x: np.ndarray, trace: bool = False):
    if "nc" not in _CACHE:
        _CACHE["nc"] = build_nc()
    nc = _CACHE["nc"]
    in_maps = [
        {"x": np.ascontiguousarray(x[i * ROWS : (i + 1) * ROWS])}
        for i in range(N_CORES)
    ]
    return run_bass_kernel_spmd(
        nc, in_maps, core_ids=list(range(N_CORES)), trace=trace
    )


def kernel(received_bits: np.ndarray):
    x = np.asarray(received_bits, dtype=np.float32)
    assert x.shape == (B, C), x.shape
    trace = bool(os.environ.get("KERNEL_TRACE"))
    res = _run(x, trace=trace)
    _CACHE["last_result"] = res
    decoded = np.concatenate([r["dec"] for r in res.results], axis=0)
    nt = ROWS // TROWS
    # ec[p, 2t+k] = error count of row t*256 + 2p + k
    ec = np.concatenate(
        [
            r["ec"].reshape(P, nt, RPP).transpose(1, 0, 2).reshape(-1)
            for r in res.results
        ],
        axis=0,
    )
    return decoded, ec


# revision 5
# speedup vs baseline: 1.9631x; 1.5322x over previous
"""Trainium2 Bass kernel: batched majority-vote (REP=2) BCH fallback decode.

Input : received_bits (65536, 2000) float32 soft bits in [0, 1)
Output: decoded (65536, 1000) float32, error_count (65536,) float32

Math (REP=2 groups of consecutive bits):
  hard = received > 0.5
  decoded[j]  = hard[2j] AND hard[2j+1]
  error_count = #{j : hard[2j] XOR hard[2j+1]}
              = sum(hard) - 2 * sum(decoded)       # identity used below

Sharding: pure data parallel over the batch across 8 NeuronCores (8192
rows per core). Per core, tiles of 256 rows (2 rows per SBUF partition,
16 KB contiguous HBM runs per partition). Per tile: one 4000-wide DVE
threshold (2x mode), per row a DVE scalar_tensor_tensor AND with accum
(per-row sum D) and an ACT activation-Identity accumulation (per-row
sum S); error counts come from one final S - 2D op. The kernel is
HBM-bandwidth-bound, so decoded is stored as uint8 (0/1, exact) and
cast to float32 on the host; input loads use the Sync HWDGE ring,
stores the Scalar ring.
"""

import os

import numpy as np

import concourse.bacc as bacc
import concourse.bass as bass
import concourse.mybir as mybir
from concourse.bass_utils import run_bass_kernel_spmd
from concourse.tile import TileContext

F32 = mybir.dt.float32
BF16 = mybir.dt.bfloat16
U8 = mybir.dt.uint8
P = 128
B = 65536
C = 2000
M = C // 2
RPP = 2
TROWS = P * RPP  # 256 rows per tile
N_CORES = 8
ROWS = B // N_CORES  # 8192 rows per core


def build_nc(rows: int = ROWS, bufs_x: int = 4, bufs_h: int = 2, bufs_d: int = 3) -> bass.Bass:
    assert rows % TROWS == 0
    nt = rows // TROWS
    ncols = nt * RPP  # accumulator columns == rows/128

    nc = bacc.Bacc("TRN2", target_bir_lowering=False, debug=False)
    x = nc.dram_tensor("x", [rows, C], F32, kind="ExternalInput")
    dec = nc.dram_tensor("dec", [rows, M], U8, kind="ExternalOutput")
    ec = nc.dram_tensor("ec", [P, ncols], F32, kind="ExternalOutput")

    # tile t, partition p  <->  rows t*256 + 2p + {0,1}
    x_t = x.rearrange("(t p two) c -> t p (two c)", p=P, two=RPP)
    dec_t = dec.rearrange("(t p two) m -> t p (two m)", p=P, two=RPP)

    with TileContext(nc) as tc:
        with (
            tc.tile_pool(name="io", bufs=2) as pool,
            tc.tile_pool(name="acc", bufs=1) as accpool,
        ):
            s_acc = accpool.tile([P, ncols], F32, name="s_acc")
            d_acc = accpool.tile([P, ncols], F32, name="d_acc")
            ec_t = accpool.tile([P, ncols], F32, name="ec_t")
            for t in range(nt):
                xt = pool.tile([P, RPP * C], F32, tag="x", name=f"x_{t}", bufs=bufs_x)
                nc.sync.dma_start(out=xt[:], in_=x_t[t])
                hard = pool.tile([P, RPP * C], F32, tag="h", name=f"h_{t}", bufs=bufs_h)
                dect = pool.tile([P, RPP * M], U8, tag="d", name=f"d_{t}", bufs=bufs_d)
                junk = pool.tile([P, RPP * C], BF16, tag="j", name=f"j_{t}", bufs=2)
                # hard = (x > 0.5) over both rows at once (contiguous, 2x mode)
                nc.vector.tensor_scalar(
                    hard[:],
                    xt[:],
                    0.5,
                    None,
                    mybir.AluOpType.is_gt,
                )
                for k in range(RPP):
                    a = t * RPP + k
                    # dec = hard_even AND hard_odd, accum -> per-row sum D (DVE)
                    nc.vector.scalar_tensor_tensor(
                        dect[:, k * M : (k + 1) * M],
                        hard[:, k * C : (k + 1) * C : 2],
                        0.0,
                        hard[:, k * C + 1 : (k + 1) * C : 2],
                        mybir.AluOpType.bypass,
                        mybir.AluOpType.logical_and,
                        accum_out=d_acc[:, a : a + 1],
                    )
                    # S_r = per-row sum of hard (ACT; elementwise out -> scratch)
                    nc.scalar.activation(
                        junk[:, k * C : (k + 1) * C],
                        hard[:, k * C : (k + 1) * C],
                        mybir.ActivationFunctionType.Identity,
                        accum_out=s_acc[:, a : a + 1],
                    )
                nc.scalar.dma_start(out=dec_t[t], in_=dect[:])
            # ec = S - 2*D
            nc.vector.scalar_tensor_tensor(
                ec_t[:],
                d_acc[:],
                -2.0,
                s_acc[:],
                mybir.AluOpType.mult,
                mybir.AluOpType.add,
            )
            nc.scalar.dma_start(out=ec[:, :], in_=ec_t[:])
    nc.compile()
    return nc


_CACHE: dict = {}


def _run(x: np.ndarray, trace: bool = False):
    if "nc" not in _CACHE:
        _CACHE["nc"] = build_nc()
    nc = _CACHE["nc"]
    in_maps = [
        {"x": np.ascontiguousarray(x[i * ROWS : (i + 1) * ROWS])}
        for i in range(N_CORES)
    ]
    return run_bass_kernel_spmd(
        nc, in_maps, core_ids=list(range(N_CORES)), trace=trace
    )


def kernel(received_bits: np.ndarray):
    x = np.asarray(received_bits, dtype=np.float32)
    assert x.shape == (B, C), x.shape
    trace = bool(os.environ.get("KERNEL_TRACE"))
    res = _run(x, trace=trace)
    _CACHE["last_result"] = res
    decoded = np.concatenate(
        [r["dec"].astype(np.float32) for r in res.results], axis=0
    )
    nt = ROWS // TROWS
    # ec[p, 2t+k] = error count of row t*256 + 2p + k
    ec = np.concatenate(
        [
            r["ec"].reshape(P, nt, RPP).transpose(1, 0, 2).reshape(-1)
            for r in res.results
        ],
        axis=0,
    )
    return decoded, ec


# revision 6
# speedup vs baseline: 1.9708x; 1.0039x over previous
"""Trainium2 Bass kernel: batched majority-vote (REP=2) BCH fallback decode.

Input : received_bits (65536, 2000) float32 soft bits in [0, 1)
Output: decoded (65536, 1000) float32, error_count (65536,) float32

Math (REP=2 groups of consecutive bits):
  hard = received > 0.5
  decoded[j]  = hard[2j] AND hard[2j+1]
  error_count = #{j : hard[2j] XOR hard[2j+1]}
              = sum(hard) - 2 * sum(decoded)       # identity used below

Sharding: pure data parallel over the batch across 8 NeuronCores (8192
rows per core). Per core, tiles of 256 rows (2 rows per SBUF partition,
16 KB contiguous HBM runs per partition). Per tile: one 4000-wide DVE
threshold (2x mode), per row a DVE scalar_tensor_tensor AND with accum
(per-row sum D) and an ACT activation-Identity accumulation (per-row
sum S); error counts come from one final S - 2D op. The kernel is
HBM-bandwidth-bound, so decoded is stored as uint8 (0/1, exact) and
cast to float32 on the host; input loads use the Sync HWDGE ring,
stores the Scalar ring.
"""

import os

import numpy as np

import concourse.bacc as bacc
import concourse.bass as bass
import concourse.mybir as mybir
from concourse.bass_utils import run_bass_kernel_spmd
from concourse.tile import TileContext

F32 = mybir.dt.float32
BF16 = mybir.dt.bfloat16
U8 = mybir.dt.uint8
P = 128
B = 65536
C = 2000
M = C // 2
RPP = 2
TROWS = P * RPP  # 256 rows per tile
N_CORES = 8
ROWS = B // N_CORES  # 8192 rows per core


def build_nc(rows: int = ROWS, bufs_x: int = 6, bufs_h: int = 2, bufs_d: int = 4) -> bass.Bass:
    assert rows % TROWS == 0
    nt = rows // TROWS
    ncols = nt * RPP  # accumulator columns == rows/128

    nc = bacc.Bacc("TRN2", target_bir_lowering=False, debug=False)
    x = nc.dram_tensor("x", [rows, C], F32, kind="ExternalInput")
    dec = nc.dram_tensor("dec", [rows, M], U8, kind="ExternalOutput")
    ec = nc.dram_tensor("ec", [P, ncols], F32, kind="ExternalOutput")

    # tile t, partition p  <->  rows t*256 + 2p + {0,1}
    x_t = x.rearrange("(t p two) c -> t p (two c)", p=P, two=RPP)
    dec_t = dec.rearrange("(t p two) m -> t p (two m)", p=P, two=RPP)

    with TileContext(nc) as tc:
        with (
            tc.tile_pool(name="io", bufs=2) as pool,
            tc.tile_pool(name="acc", bufs=1) as accpool,
        ):
            s_acc = accpool.tile([P, ncols], F32, name="s_acc")
            d_acc = accpool.tile([P, ncols], F32, name="d_acc")
            ec_t = accpool.tile([P, ncols], F32, name="ec_t")
            for t in range(nt):
                xt = pool.tile([P, RPP * C], F32, tag="x", name=f"x_{t}", bufs=bufs_x)
                nc.sync.dma_start(out=xt[:], in_=x_t[t])
                hard = pool.tile([P, RPP * C], F32, tag="h", name=f"h_{t}", bufs=bufs_h)
                dect = pool.tile([P, RPP * M], U8, tag="d", name=f"d_{t}", bufs=bufs_d)
                junk = pool.tile([P, RPP * C], BF16, tag="j", name=f"j_{t}", bufs=2)
                # hard = (x > 0.5) over both rows at once (contiguous, 2x mode)
                nc.vector.tensor_scalar(
                    hard[:],
                    xt[:],
                    0.5,
                    None,
                    mybir.AluOpType.is_gt,
                )
                for k in range(RPP):
                    a = t * RPP + k
                    # dec = hard_even AND hard_odd, accum -> per-row sum D (DVE)
                    nc.vector.scalar_tensor_tensor(
                        dect[:, k * M : (k + 1) * M],
                        hard[:, k * C : (k + 1) * C : 2],
                        0.0,
                        hard[:, k * C + 1 : (k + 1) * C : 2],
                        mybir.AluOpType.bypass,
                        mybir.AluOpType.logical_and,
                        accum_out=d_acc[:, a : a + 1],
                    )
                    # S_r = per-row sum of hard (ACT; elementwise out -> scratch)
                    nc.scalar.activation(
                        junk[:, k * C : (k + 1) * C],
                        hard[:, k * C : (k + 1) * C],
                        mybir.ActivationFunctionType.Identity,
                        accum_out=s_acc[:, a : a + 1],
                    )
                nc.scalar.dma_start(out=dec_t[t], in_=dect[:])
            # ec = S - 2*D
            nc.vector.scalar_tensor_tensor(
                ec_t[:],
                d_acc[:],
                -2.0,
                s_acc[:],
                mybir.AluOpType.mult,
                mybir.AluOpType.add,
            )
            nc.scalar.dma_start(out=ec[:, :], in_=ec_t[:])
    nc.compile()
    return nc


_CACHE: dict = {}


def _run(x: np.ndarray, trace: bool = False):
    if "nc" not in _CACHE:
        _CACHE["nc"] = build_nc()
    nc = _CACHE["nc"]
    in_maps = [
        {"x": np.ascontiguousarray(x[i * ROWS : (i + 1) * ROWS])}
        for i in range(N_CORES)
    ]
    return run_bass_kernel_spmd(
        nc, in_maps, core_ids=list(range(N_CORES)), trace=trace
    )


def kernel(received_bits: np.ndarray):
    x = np.asarray(received_bits, dtype=np.float32)
    assert x.shape == (B, C), x.shape
    trace = bool(os.environ.get("KERNEL_TRACE"))
    res = _run(x, trace=trace)
    _CACHE["last_result"] = res
    decoded = np.concatenate(
        [r["dec"].astype(np.float32) for r in res.results], axis=0
    )
    nt = ROWS // TROWS
    # ec[p, 2t+k] = error count of row t*256 + 2p + k
    ec = np.concatenate(
        [
            r["ec"].reshape(P, nt, RPP).transpose(1, 0, 2).reshape(-1)
            for r in res.results
        ],
        axis=0,
    )
    return decoded, ec
